# revision 5
# baseline (speedup 1.0000x reference)
"""Trainium2 Bass kernel for a dense transformer block (B=4, T=2048, C=1024,
H=16 heads, D_FF=4096) on 8 NeuronCores.

Sharding: zero-collective row sharding. Each batch row b is handled by two
cores: core (2b+half) computes half of the T=2048 rows, split
causally-balanced by 128-row blocks (half 0: blocks 0-3 and 12-15; half 1:
blocks 4-11) so both halves cover the same number of (query, key) block
pairs. K/V are computed redundantly for the full batch on both cores of a
batch (cheaper than an all-reduce); everything downstream is row-local and
the host reassembles the output. The two halves need different causal loop
structures, so two programs are built and dispatched concurrently on
disjoint 4-device meshes (cores 0-3 run half 0, cores 4-7 run half 1).

Numerics: matmuls in bf16 with f32 PSUM accumulation; LayerNorm in f32.
LN scales are folded host-side into adjacent weight matrices; LN biases fold
into host-computed per-column constants (exact for unmasked rows; masked
rows are zeroed by the final padding mask anyway). Softmax runs unnormalized
in transposed [key, query] layout: exp on the ACT engine with the 1/32 score
scale fused in, causal + key-padding masks applied multiplicatively, the
denominator taken from a ones-column appended to V, and normalization
applied as a per-partition reciprocal multiply.
"""

import numpy as np
import ml_dtypes

import concourse.bass as bass
import concourse.tile as tile
from concourse import mybir
from concourse.masks import make_identity
from concourse.vector_clock import ScopedClock

# ---------------------------------------------------------------------------
# Patch: walrus TPB_CTRL codegen accepts at most one sem wait per instruction;
# TileContext's exit drain carries one wait per logical proc. Split the waits
# across a chain of drains (SP runs them serially; the all-engine barrier that
# follows still observes everything complete).


def _drain_and_barrier(self, tick_clock, wait_clock):
    drain_inst = self.nc.sync.drain()
    wait_clock.add_sem_waits(
        drain_inst.ins, ScopedClock({None: tick_clock.global_clock})
    )
    ins = drain_inst.ins
    si = ins.sync_info
    waits = list(si.on_wait) if si is not None else []
    if si is not None and len(waits) > 1:
        SyncInfo = type(si)
        ins.sync_info = SyncInfo(on_wait=waits[:1], on_update=list(si.on_update))
        for w in waits[1:]:
            extra = self.nc.sync.drain()
            extra.ins.sync_info = SyncInfo(on_wait=[w], on_update=[])
    self.nc.all_engine_barrier()
    popped = self.nc._tile_sem_poison_stack.pop()
    assert popped is self._sem_poison
    self.nc.clear_and_free_semaphores(list(self.sems.allocated().values()))
    self.nc.all_engine_barrier()


tile.TileContext._drain_and_barrier = _drain_and_barrier

# Same walrus limit applies to every engine instruction: hoist all but the
# last sem wait of each scheduled instruction onto single-wait NOPs inserted
# just before it on the same engine (engines execute serially, so the chain
# of waits is equivalent). DMA descriptors go through DGE structs, which
# accept their waits, so only engine instructions are rewritten.
_ENGINE_TYPES = {
    mybir.EngineType.PE, mybir.EngineType.DVE, mybir.EngineType.Activation,
    mybir.EngineType.Pool, mybir.EngineType.SP,
}

_orig_lower_ordered = tile.TileContext._lower_ordered_insts


def _split_multi_waits(ordered):
    for bb_name, insts in ordered.items():
        out = []
        for inst in insts:
            si = getattr(inst, "sync_info", None)
            eng = getattr(inst, "engine", None)
            if (si is not None and len(si.on_wait) > 1
                    and eng in _ENGINE_TYPES):
                SyncInfo = type(si)
                waits = list(si.on_wait)
                for k, w in enumerate(waits[:-1]):
                    out.append(mybir.InstNoOp(
                        name=f"{inst.name}w{k}",
                        sync_info=mybir.SyncInfo(on_wait=[w], on_update=[]),
                        engine=eng,
                        bass_nofuse=True,
                    ))
                inst.sync_info = SyncInfo(
                    on_wait=waits[-1:], on_update=list(si.on_update))
            out.append(inst)
        insts[:] = out


def _lower_ordered_insts(self, ordered):
    _split_multi_waits(ordered)
    return _orig_lower_ordered(self, ordered)


tile.TileContext._lower_ordered_insts = _lower_ordered_insts

# ---------------------------------------------------------------------------

F32 = mybir.dt.float32
BF16 = mybir.dt.bfloat16
AF = mybir.ActivationFunctionType
OP = mybir.AluOpType

B, T, C = 4, 2048, 1024
H, HS = 16, 64
DFF = 4 * C
EPS = 1e-5
NB = T // 128           # 16 row blocks per batch
P = 128

# own row blocks (global block ids) per half -- causally balanced
OWN_BLOCKS = [
    [0, 1, 2, 3, 12, 13, 14, 15],
    [4, 5, 6, 7, 8, 9, 10, 11],
]
# own 512-wide t-chunks (global chunk ids, chunk = 4 blocks) per half
OWN_TCHUNKS = [[0, 3], [1, 2]]


def _layernorm_to_bf16(nc, pool, x_tile, m_col, out_bf16, eps_ap):
    """LN over the free axis (1024) of x_tile [128,1024] f32, scaled by the
    per-row mask column m_col [128,1], written to out_bf16 [128,1024]."""
    stats = pool.tile([P, 2, 6], F32, tag="ln_stats")
    nc.vector.bn_stats(out=stats[:, 0, :], in_=x_tile[:, 0:512])
    nc.vector.bn_stats(out=stats[:, 1, :], in_=x_tile[:, 512:1024])
    mv = pool.tile([P, 2], F32, tag="ln_mv")
    nc.vector.bn_aggr(out=mv, in_=stats)
    rstd = pool.tile([P, 1], F32, tag="ln_rstd")
    nc.scalar.activation(out=rstd, in_=mv[:, 1:2], func=AF.Sqrt, bias=eps_ap)
    nc.vector.reciprocal(out=rstd, in_=rstd)
    nc.vector.tensor_mul(rstd, rstd, m_col)
    nc.vector.tensor_scalar(
        out=out_bf16, in0=x_tile,
        scalar1=mv[:, 0:1], scalar2=rstd,
        op0=OP.subtract, op1=OP.mult,
    )


def build_nc(half: int, n_iters: int = 1, use_pb=False, use_b2=False):
    """Build the per-core program for row-half `half` (0 or 1)."""
    own = OWN_BLOCKS[half]
    own_tch = OWN_TCHUNKS[half]
    nc = bass.Bass()

    x_d = nc.declare_dram_parameter("x", [T, C], F32, isOutput=False)
    mT_d = nc.declare_dram_parameter("mT", [P, NB], F32, isOutput=False)
    wq_d = nc.declare_dram_parameter("wq", [C, C], BF16, isOutput=False)
    wk_d = nc.declare_dram_parameter("wk", [C, C], BF16, isOutput=False)
    wv_d = nc.declare_dram_parameter("wv", [C, C], BF16, isOutput=False)
    pw_d = nc.declare_dram_parameter("pw", [C, C], BF16, isOutput=False)
    w1_d = nc.declare_dram_parameter("w1", [C, DFF], BF16, isOutput=False)
    b1_d = nc.declare_dram_parameter("b1T", [P, DFF // P], F32, isOutput=False)
    w2_d = nc.declare_dram_parameter("w2", [DFF, C], BF16, isOutput=False)
    pb_d = nc.declare_dram_parameter("pb", [1, C], F32, isOutput=False)
    b2_d = nc.declare_dram_parameter("b2", [1, C], F32, isOutput=False)
    y_d = nc.declare_dram_parameter("y", [8 * P, C], F32, isOutput=True)

    with tile.TileContext(nc) as tc, \
         tc.tile_pool(name="consts", bufs=1) as consts, \
         tc.tile_pool(name="persist", bufs=1) as persist:

        ident = consts.tile([P, P], BF16, tag="ident")
        make_identity(nc, ident)
        eps_ap = consts.tile([P, 1], F32, tag="eps")
        nc.vector.memset(eps_ap, EPS)
        mT = consts.tile([P, NB], F32, tag="mT")
        nc.sync.dma_start(out=mT, in_=mT_d[:, :])
        ones_r = consts.tile([1, P], F32, tag="ones_r")
        nc.vector.memset(ones_r, 1.0)
        pb_t = consts.tile([1, C], F32, tag="pb")
        if use_pb:
            nc.sync.dma_start(out=pb_t, in_=pb_d[:, :])
        b2_t = consts.tile([1, C], F32, tag="b2")
        if use_b2:
            nc.sync.dma_start(out=b2_t, in_=b2_d[:, :])
        b1T = consts.tile([P, DFF // P], F32, tag="b1T")
        nc.sync.dma_start(out=b1T, in_=b1_d[:, :])
        # multiplicative causal masks for the 4 diagonal s-blocks of each
        # 512-wide t-chunk: caus[j][s, t] = 1.0 if t >= s + 128*j else 0
        caus = []
        for j in range(4):
            cm = consts.tile([P, 512], BF16, tag=f"caus{j}")
            nc.gpsimd.memset(cm, 1.0)
            nc.gpsimd.affine_select(
                out=cm, in_=cm,
                compare_op=OP.is_ge,
                fill=0.0, base=-128 * j,
                pattern=[[1, 512]], channel_multiplier=-1,
            )
            caus.append(cm)

        for _ in range(n_iters):
            # -------- Phase A: LN1 + transpose -> hT [C-part, row] --------
            hT = [persist.tile([P, T], BF16, tag=f"hT{cb}", name=f"hT{cb}") for cb in range(8)]
            with tc.tile_pool(name="ph_a", bufs=3) as pa, \
                 tc.tile_pool(name="ps_a", bufs=4, space="PSUM") as psa:
                for rb in range(NB):
                    xt = pa.tile([P, C], F32, tag="x_stream")
                    nc.sync.dma_start(out=xt, in_=x_d[rb * P:(rb + 1) * P, :])
                    h = pa.tile([P, C], BF16, tag="h")
                    _layernorm_to_bf16(nc, pa, xt, mT[:, rb:rb + 1], h, eps_ap)
                    for cb in range(8):
                        pt = psa.tile([P, P], BF16, tag="tr")
                        nc.tensor.transpose(pt, h[:, cb * P:(cb + 1) * P], ident)
                        nc.vector.tensor_copy(
                            out=hT[cb][:, rb * P:(rb + 1) * P], in_=pt)

            # -------- Phase B: QKV projections --------
            QT = [persist.tile([P, 8 * P], BF16, tag=f"QT{mb}", name=f"QT{mb}") for mb in range(8)]
            KT = [persist.tile([P, T], BF16, tag=f"KT{mb}", name=f"KT{mb}") for mb in range(8)]
            V = [persist.tile([P, H, 65], BF16, tag=f"V{sb}", name=f"V{sb}") for sb in range(NB)]
            with tc.tile_pool(name="ph_b", bufs=2) as pbp, \
                 tc.tile_pool(name="ps_b", bufs=4, space="PSUM") as psb:
                for sb in range(NB):
                    nc.vector.memset(V[sb][:, :, 64:65], 1.0)
                for w_d, full in ((wq_d, False), (wk_d, True)):
                    wt = [pbp.tile([P, C], BF16, tag=f"w{kb}", name=f"w{kb}") for kb in range(8)]
                    for kb in range(8):
                        nc.sync.dma_start(
                            out=wt[kb], in_=w_d[kb * P:(kb + 1) * P, :])
                    nchunks = list(range(4)) if full else own_tch
                    for mb in range(8):
                        for ic, nch in enumerate(nchunks):
                            ps = psb.tile([P, 512], F32, tag="mm")
                            for kb in range(8):
                                nc.tensor.matmul(
                                    ps,
                                    lhsT=wt[kb][:, mb * P:(mb + 1) * P],
                                    rhs=hT[kb][:, nch * 512:(nch + 1) * 512],
                                    start=(kb == 0), stop=(kb == 7),
                                )
                            dst = KT[mb] if full else QT[mb]
                            off = (nch if full else ic) * 512
                            nc.vector.tensor_copy(out=dst[:, off:off + 512], in_=ps)
                wt = [pbp.tile([P, C], BF16, tag=f"w{kb}", name=f"w{kb}") for kb in range(8)]
                for kb in range(8):
                    nc.sync.dma_start(out=wt[kb], in_=wv_d[kb * P:(kb + 1) * P, :])
                for sb in range(NB):
                    for nch in range(2):
                        ps = psb.tile([P, 512], F32, tag="mm")
                        for kb in range(8):
                            nc.tensor.matmul(
                                ps,
                                lhsT=hT[kb][:, sb * P:(sb + 1) * P],
                                rhs=wt[kb][:, nch * 512:(nch + 1) * 512],
                                start=(kb == 0), stop=(kb == 7),
                            )
                        nc.vector.tensor_copy(
                            out=V[sb][:, nch * 8:(nch + 1) * 8, 0:64],
                            in_=ps.rearrange("p (h d) -> p h d", d=64),
                        )

            # -------- Phase C: attention -> AOT [C-part, row] --------
            AOT = [persist.tile([P, 8 * P], BF16, tag=f"AOT{cb}", name=f"AOT{cb}") for cb in range(8)]
            with tc.tile_pool(name="ph_c", bufs=4) as pc, \
                 tc.tile_pool(name="ps_s", bufs=2, space="PSUM") as pss, \
                 tc.tile_pool(name="ps_o", bufs=1, space="PSUM") as pso, \
                 tc.tile_pool(name="ps_ct", bufs=2, space="PSUM") as psct:
                for ic, tch in enumerate(own_tch):
                    n_sb = (tch + 1) * 4      # causal: s-blocks 0 .. tch*4+3
                    for h in range(H):
                        ht, hp = h // 2, (h % 2) * 64
                        o_ps = [pso.tile([P, 65], F32, tag=f"o{tb}", name=f"o{tb}")
                                for tb in range(4)]
                        for sb in range(n_sb):
                            s_ps = pss.tile([P, 512], F32, tag="s")
                            nc.tensor.matmul(
                                s_ps,
                                lhsT=KT[ht][hp:hp + 64, sb * P:(sb + 1) * P],
                                rhs=QT[ht][hp:hp + 64, ic * 512:(ic + 1) * 512],
                                start=True, stop=True,
                            )
                            e = pc.tile([P, 512], BF16, tag="e")
                            nc.scalar.activation(
                                out=e, in_=s_ps, func=AF.Exp, scale=C ** -0.5)
                            j = sb - tch * 4
                            if j >= 0:
                                nc.vector.tensor_mul(e, e, caus[j])
                            nc.vector.tensor_scalar_mul(e, e, mT[:, sb:sb + 1])
                            for tb in range(4):
                                if sb > tch * 4 + tb:
                                    continue  # E block is entirely zero
                                nc.tensor.matmul(
                                    o_ps[tb],
                                    lhsT=e[:, tb * P:(tb + 1) * P],
                                    rhs=V[sb][:, h, :],
                                    start=(sb == 0),
                                    stop=(sb == tch * 4 + tb),
                                )
                        for tb in range(4):
                            rec = pc.tile([P, 1], F32, tag="rec")
                            nc.vector.reciprocal(rec, o_ps[tb][:, 64:65])
                            tmp = pc.tile([P, 64], BF16, tag="otmp")
                            nc.vector.tensor_scalar_mul(
                                tmp, o_ps[tb][:, 0:64], rec)
                            pt = psct.tile([64, P], BF16, tag="tr")
                            nc.tensor.transpose(pt, tmp, ident)
                            nc.vector.tensor_copy(
                                out=AOT[ht][hp:hp + 64,
                                            (ic * 4 + tb) * P:(ic * 4 + tb + 1) * P],
                                in_=pt)

            # ------ Phase D: proj + residual + LN2 + transpose -> h2T ------
            X1 = [persist.tile([P, C], F32, tag=f"KT{ir}", name=f"X1_{ir}") for ir in range(8)]
            h2T = [persist.tile([P, 8 * P], BF16, tag=f"V{cb}", name=f"h2T{cb}") for cb in range(8)]
            with tc.tile_pool(name="ph_d", bufs=3) as pd, \
                 tc.tile_pool(name="pw_pool", bufs=1) as pwp, \
                 tc.tile_pool(name="ps_d", bufs=2, space="PSUM") as psd, \
                 tc.tile_pool(name="ps_dt", bufs=4, space="PSUM") as psdt:
                pwt = [pwp.tile([P, C], BF16, tag=f"pw{kb}", name=f"pw{kb}") for kb in range(8)]
                for kb in range(8):
                    nc.sync.dma_start(out=pwt[kb], in_=pw_d[kb * P:(kb + 1) * P, :])
                for ir, rb in enumerate(own):
                    xr = pd.tile([P, C], F32, tag="xr")
                    nc.sync.dma_start(out=xr, in_=x_d[rb * P:(rb + 1) * P, :])
                    x1 = X1[ir]
                    for nch in range(2):
                        ps = psd.tile([P, 512], F32, tag="mm")
                        for kb in range(8):
                            nc.tensor.matmul(
                                ps,
                                lhsT=AOT[kb][:, ir * P:(ir + 1) * P],
                                rhs=pwt[kb][:, nch * 512:(nch + 1) * 512],
                                start=(kb == 0),
                                stop=(kb == 7 and not use_pb),
                            )
                        if use_pb:
                            nc.tensor.matmul(
                                ps, lhsT=ones_r,
                                rhs=pb_t[0:1, nch * 512:(nch + 1) * 512],
                                start=False, stop=True,
                            )
                        nc.vector.tensor_add(
                            x1[:, nch * 512:(nch + 1) * 512], ps,
                            xr[:, nch * 512:(nch + 1) * 512])
                    h2 = pd.tile([P, C], BF16, tag="h2")
                    _layernorm_to_bf16(nc, pd, x1, mT[:, rb:rb + 1], h2, eps_ap)
                    for cb in range(8):
                        pt = psdt.tile([P, P], BF16, tag="tr")
                        nc.tensor.transpose(pt, h2[:, cb * P:(cb + 1) * P], ident)
                        nc.vector.tensor_copy(
                            out=h2T[cb][:, ir * P:(ir + 1) * P], in_=pt)

            # -------- Phase E: FFN --------
            # FFT[mb] [128, 1024] bf16 lives in slots freed by hT/QT/AOT
            fft_tiles = {}
            for q in range(8):
                fft_tiles[q] = persist.tile([P, T], BF16, tag=f"hT{q}", name=f"fftp{q}")
            for q in range(8):
                fft_tiles[8 + q] = persist.tile([P, 8 * P], BF16, tag=f"QT{q}", name=f"ffts{q}")
            for q in range(8):
                fft_tiles[16 + q] = persist.tile([P, 8 * P], BF16, tag=f"AOT{q}", name=f"ffta{q}")

            def fft_ap(mb):
                if mb < 16:
                    return fft_tiles[mb // 2][:, (mb % 2) * 1024:(mb % 2 + 1) * 1024]
                return fft_tiles[mb - 8][:, :]

            with tc.tile_pool(name="ph_e", bufs=3) as pe, \
                 tc.tile_pool(name="ps_e1", bufs=4, space="PSUM") as pse1, \
                 tc.tile_pool(name="ps_e2", bufs=1, space="PSUM") as pse2:
                for mbg in range(8):       # groups of 4 ff blocks
                    w1g = [pe.tile([P, 512], BF16, tag=f"w1g{kb}", name=f"w1g{kb}")
                           for kb in range(8)]
                    for kb in range(8):
                        nc.sync.dma_start(
                            out=w1g[kb],
                            in_=w1_d[kb * P:(kb + 1) * P,
                                     mbg * 512:(mbg + 1) * 512])
                    for ml in range(4):
                        mb = mbg * 4 + ml
                        for nch in range(2):
                            ps = pse1.tile([P, 512], F32, tag="mm")
                            for kb in range(8):
                                nc.tensor.matmul(
                                    ps,
                                    lhsT=w1g[kb][:, ml * P:(ml + 1) * P],
                                    rhs=h2T[kb][:, nch * 512:(nch + 1) * 512],
                                    start=(kb == 0), stop=(kb == 7),
                                )
                            nc.scalar.activation(
                                out=fft_ap(mb)[:, nch * 512:(nch + 1) * 512],
                                in_=ps, func=AF.Relu, bias=b1T[:, mb:mb + 1])
                for rg in range(2):        # groups of 4 own row blocks
                    for nch in range(2):
                        ps_list = [pse2.tile([P, 512], F32, tag=f"m{rl}", name=f"m{rl}")
                                   for rl in range(4)]
                        for mb in range(DFF // P):
                            w2t = pe.tile([P, 512], BF16, tag="w2t")
                            nc.sync.dma_start(
                                out=w2t,
                                in_=w2_d[mb * P:(mb + 1) * P,
                                         nch * 512:(nch + 1) * 512])
                            for rl in range(4):
                                ir = rg * 4 + rl
                                nc.tensor.matmul(
                                    ps_list[rl],
                                    lhsT=fft_ap(mb)[:, ir * P:(ir + 1) * P],
                                    rhs=w2t,
                                    start=(mb == 0),
                                    stop=(mb == DFF // P - 1 and not use_b2),
                                )
                        for rl in range(4):
                            if use_b2:
                                nc.tensor.matmul(
                                    ps_list[rl], lhsT=ones_r,
                                    rhs=b2_t[0:1, nch * 512:(nch + 1) * 512],
                                    start=False, stop=True,
                                )
                            ir = rg * 4 + rl
                            rb = own[ir]
                            o = pe.tile([P, 512], F32, tag="out")
                            nc.vector.tensor_add(
                                o, ps_list[rl],
                                X1[ir][:, nch * 512:(nch + 1) * 512])
                            nc.vector.tensor_scalar_mul(o, o, mT[:, rb:rb + 1])
                            nc.sync.dma_start(
                                out=y_d[ir * P:(ir + 1) * P,
                                        nch * 512:(nch + 1) * 512],
                                in_=o)
    return nc


# ---------------------------------------------------------------------------
# Host side


def _prep_inputs(inputs):
    """Fold LN scales/biases into weights; cast to bf16; build per-batch
    tensors shared by both cores of a batch."""
    f32 = np.float32
    x = np.asarray(inputs["x"], f32)                      # [B,T,C]
    kpm = np.asarray(inputs["key_padding_mask"])          # [B,T] bool
    wq = np.asarray(inputs["wq"], f32).reshape(C, C)
    wk = np.asarray(inputs["wk"], f32).reshape(C, C)
    wv = np.asarray(inputs["wv"], f32).reshape(C, C)
    pw = np.asarray(inputs["proj_w"], f32)
    pb = np.asarray(inputs["proj_b"], f32)
    w1 = np.asarray(inputs["ff_w1"], f32)
    b1 = np.asarray(inputs["ff_b1"], f32)
    w2 = np.asarray(inputs["ff_w2"], f32)
    b2 = np.asarray(inputs["ff_b2"], f32)
    ln1w = np.asarray(inputs["ln1_w"], f32)
    ln1b = np.asarray(inputs["ln1_b"], f32)
    ln2w = np.asarray(inputs["ln2_w"], f32)
    ln2b = np.asarray(inputs["ln2_b"], f32)

    assert not (np.any(ln1b) or False), \
        "nonzero ln1_b is not supported by this kernel build"

    bf = ml_dtypes.bfloat16
    shared = dict(
        wq=np.ascontiguousarray((ln1w[:, None] * wq).astype(bf)),
        wk=np.ascontiguousarray((ln1w[:, None] * wk).astype(bf)),
        wv=np.ascontiguousarray((ln1w[:, None] * wv).astype(bf)),
        pw=np.ascontiguousarray(pw.astype(bf)),
        w1=np.ascontiguousarray((ln2w[:, None] * w1).astype(bf)),
        b1T=np.ascontiguousarray((b1 + ln2b @ w1).reshape(DFF // P, P).T
                                 .astype(f32)),
        w2=np.ascontiguousarray(w2.astype(bf)),
        pb=np.ascontiguousarray(pb.reshape(1, C).astype(f32)),
        b2=np.ascontiguousarray(b2.reshape(1, C).astype(f32)),
    )
    per_batch = []
    for b in range(B):
        m = kpm[b].astype(f32)
        per_batch.append(dict(
            x=np.ascontiguousarray(x[b]),
            mT=np.ascontiguousarray(m.reshape(NB, P).T),
        ))
    use_pb = bool(np.any(pb))
    use_b2 = bool(np.any(b2))
    return shared, per_batch, use_pb, use_b2


def make_executable(nc, devices):
    """Build a jitted shard_map executable for `nc` over `devices` (one core
    per device). Returns (fn, in_names, out_names, out_avals)."""
    import jax
    from jax.sharding import Mesh, PartitionSpec
    from jax.experimental.shard_map import shard_map
    from concourse import bass2jax

    bass2jax.install_neuronx_cc_hook()
    partition_name = nc.partition_id_tensor.name if nc.partition_id_tensor else None
    in_names, out_names, out_avals, zero_outs = [], [], [], []
    for alloc in nc.m.functions[0].allocations:
        if not isinstance(alloc, mybir.MemoryLocationSet):
            continue
        name = alloc.memorylocations[0].name
        if alloc.kind == "ExternalInput":
            if name != partition_name:
                in_names.append(name)
        elif alloc.kind == "ExternalOutput":
            out_names.append(name)
            shape = tuple(alloc.tensor_shape)
            dtype = mybir.dt.np(alloc.dtype)
            out_avals.append(jax.core.ShapedArray(shape, dtype))
            zero_outs.append(np.zeros(shape, dtype))
    n_params = len(in_names)
    all_names = list(in_names) + list(out_names)
    if partition_name is not None:
        all_names.append(partition_name)

    def _body(*args):
        operands = list(args)
        if partition_name is not None:
            operands.append(bass2jax.partition_id_tensor())
        outs = bass2jax._bass_exec_p.bind(
            *operands,
            out_avals=tuple(out_avals),
            in_names=tuple(all_names),
            out_names=tuple(out_names),
            lowering_input_output_aliases=(),
            sim_require_finite=True,
            sim_require_nnan=True,
            nc=nc,
        )
        return tuple(outs)

    n = len(devices)
    mesh = Mesh(np.asarray(devices), ("core",))
    n_outs = len(out_names)
    fn = jax.jit(
        shard_map(_body, mesh=mesh,
                  in_specs=(PartitionSpec("core"),) * (n_params + n_outs),
                  out_specs=(PartitionSpec("core"),) * n_outs,
                  check_rep=False),
        keep_unused=True,
    )

    def run(in_maps):
        assert len(in_maps) == n
        concat_in = [
            np.concatenate([np.asarray(in_maps[c][k]) for c in range(n)], axis=0)
            for k in in_names
        ]
        concat_zeros = [
            np.zeros((n * z.shape[0], *z.shape[1:]), z.dtype) for z in zero_outs
        ]
        outs = fn(*concat_in, *concat_zeros)
        return [
            {name: np.asarray(outs[i]).reshape(n, *out_avals[i].shape)[c]
             for i, name in enumerate(out_names)}
            for c in range(n)
        ]

    return fn, run, in_names, out_names, out_avals, zero_outs


_EXEC_CACHE = {}


def _get_executables(n_iters=1, use_pb=False, use_b2=False):
    import jax
    key = (n_iters, use_pb, use_b2)
    if key not in _EXEC_CACHE:
        devices = jax.devices()
        assert len(devices) >= 8
        execs = []
        for half in (0, 1):
            nc = build_nc(half, n_iters=n_iters, use_pb=use_pb, use_b2=use_b2)
            execs.append(
                make_executable(nc, devices[4 * half:4 * half + 4]))
        _EXEC_CACHE[key] = execs
    return _EXEC_CACHE[key]


def run_halves(per_core_maps, n_iters=1, use_pb=False, use_b2=False):
    """per_core_maps: list of 8 in_maps (core 2b+half -> batch b, half).
    Dispatches both halves concurrently, returns per-core outputs."""
    import jax
    execs = _get_executables(n_iters, use_pb, use_b2)
    pend = []
    for half in (0, 1):
        fn, run, in_names, out_names, out_avals, zero_outs = execs[half]
        maps = [per_core_maps[4 * half + b] for b in range(B)]
        concat_in = [
            np.concatenate([np.asarray(maps[c][k]) for c in range(B)], axis=0)
            for k in in_names
        ]
        concat_zeros = [
            np.zeros((B * z.shape[0], *z.shape[1:]), z.dtype) for z in zero_outs
        ]
        pend.append((fn(*concat_in, *concat_zeros), out_names, out_avals))
    results = []
    for outs, out_names, out_avals in pend:
        jax.block_until_ready(outs)
        results.append([
            {name: np.asarray(outs[i]).reshape(B, *out_avals[i].shape)[c]
             for i, name in enumerate(out_names)}
            for c in range(B)
        ])
    return results


def kernel(**inputs):
    shared, per_batch, use_pb, use_b2 = _prep_inputs(inputs)
    per_core = []
    for half in (0, 1):
        for b in range(B):
            per_core.append({**shared, **per_batch[b]})
    results = run_halves(per_core, n_iters=1, use_pb=use_pb, use_b2=use_b2)
    y = np.empty((B, T, C), np.float32)
    for half in (0, 1):
        for b in range(B):
            yb = results[half][b]["y"]
            for ir, rb in enumerate(OWN_BLOCKS[half]):
                y[b, rb * P:(rb + 1) * P, :] = yb[ir * P:(ir + 1) * P]
    return y
